# revision 21
# baseline (speedup 1.0000x reference)
"""ConvLSTM (B=4, T=8, C=HID=256, H=W=32, 3x3 SAME convs) on 8 TRN2 cores,
computed with Winograd F(2,2) in fp16 (f32 PSUM accumulation).

Sharding: (batch 4) x (hidden-channel half 2). Each core computes 512 of the
1024 gate channels (its 128-hid slice of each gate i,f,o,g) over the full
32x32 image; the pair of cores exchanges fp16 h halves each step via a
2-rank HBM AllGather.

Compute per step per core: 256 matmuls [128ic x 128oc] x 256 tiles
(16 Winograd positions x 4 gate octiles x 2 ic-tiles x 2 convs), fp16 at
1 cyc/row, f32 PSUM. Winograd transforms run on DVE/Pool/Act:
  - inverse (A^T M A): Act evacuates 2 of 4 PSUM position-rows to fp16;
    DVE does the mixed PSUM+SBUF adds, Pool the pure-SBUF ones (Pool
    cannot read PSUM); col stage on DVE in fp16.
  - forward (B^T h B): h kept in 2x2-quad layout so the row transform is
    band-shifted contiguous adds; col stage reads a guard-column R buffer.
x is transformed to the Winograd domain host-side (data prep, like im2col).
"""
import numpy as np
from contextlib import ExitStack

import concourse.bass as bass
import concourse.tile as tile
from concourse import bacc, mybir
from concourse.bass_utils import run_bass_kernel_spmd

F16 = mybir.dt.float16
F32 = mybir.dt.float32
AF = mybir.ActivationFunctionType
ALU = mybir.AluOpType

N_CORES = 8
T = 8
RG = [[0, 1], [2, 3], [4, 5], [6, 7]]
GI, GF, GO, GG = 0, 1, 2, 3  # gate order in weights/bias: [i, f, o, g]

_cache = {}

_BT = np.array([[1, 0, -1, 0], [0, 1, 1, 0], [0, -1, 1, 0], [0, 1, 0, -1]],
               dtype=np.float64)
_G = np.array([[1, 0, 0], [.5, .5, .5], [.5, -.5, .5], [0, 0, 1]],
              dtype=np.float64)


def _build_nc():
    nc = bacc.Bacc("TRN2", target_bir_lowering=False, debug=False,
                   num_devices=N_CORES)
    w_d = nc.dram_tensor("w", [64, 128, 512], F16, kind="ExternalInput").ap()
    x_d = nc.dram_tensor("xv", [T, 128, 8192], F16, kind="ExternalInput").ap()
    b_d = nc.dram_tensor("bias", [128, 4], F32, kind="ExternalInput").ap()
    o_d = nc.dram_tensor("ho", [128, 1024], F32, kind="ExternalOutput").ap()

    with tile.TileContext(nc) as tc, ExitStack() as ctx:
        wp = ctx.enter_context(tc.tile_pool(name="wp", bufs=1))
        xp = ctx.enter_context(tc.tile_pool(name="xp", bufs=2))
        vp = ctx.enter_context(tc.tile_pool(name="vp", bufs=2))
        rp = ctx.enter_context(tc.tile_pool(name="rp", bufs=2))
        yp = ctx.enter_context(tc.tile_pool(name="yp", bufs=2))
        gp = ctx.enter_context(tc.tile_pool(name="gp", bufs=1))
        ep = ctx.enter_context(tc.tile_pool(name="ep", bufs=4))
        sp = ctx.enter_context(tc.tile_pool(name="sp", bufs=2))
        cp = ctx.enter_context(tc.tile_pool(name="cp", bufs=1))
        dp = ctx.enter_context(tc.tile_pool(name="dp", bufs=2, space="DRAM"))
        pp = ctx.enter_context(tc.tile_pool(name="pp", bufs=2, space="PSUM"))

        bt = cp.tile([128, 4], F32, tag="bias")
        nc.sync.dma_start(bt[:], b_d[:])

        ws = [wp.tile([128, 512], F16, tag=f"w{j}", name=f"w{j}")
              for j in range(64)]
        # x-conv weights stream up front (first-use pc-major order); h-conv
        # weights are deferred until after the t=0 exchange is issued so the
        # first AllReduce doesn't contend with them for SDMA engines.
        for pc in range(4):
            for pr in range(4):
                for u in range(2):
                    j = (4 * pr + pc) * 2 + u
                    nc.sync.dma_start(ws[j][:], w_d[j])

        c_t = cp.tile([128, 1024], F16, tag="c")

        # R buffers: zero both rotating buffers once; step writes never touch
        # the guard column (tc=16), so it stays 0.
        for _ in range(2):
            r0 = rp.tile([128, 4352], F16, tag="R")
            nc.vector.memset(r0[:], 0.0)

        def emit_R(Rv, u, j, hsrc, eng):
            """B^T row transform of one h-quad buffer into R[u, :, j].
            j0 plane at cols 0..15 (guard 16); j1 at cols 1..16 (guard 0)."""
            hv = hsrc[:].rearrange("p (q b c) -> p q b c", q=4, b=16, c=16)
            co = slice(j, 16 + j)
            # R0[b] = odd[b-1] - odd[b]  (b>=1);  R0[0] = -odd[0]
            eng.tensor_sub(Rv[:, u, 0, j, 1:16, co],
                           hv[:, 2 + j, 0:15, :], hv[:, 2 + j, 1:16, :])
            eng.tensor_scalar_mul(Rv[:, u, 0, j, 0, co],
                                  hv[:, 2 + j, 0, :], -1.0)
            # R1[b] = even[b] + odd[b];  R2[b] = odd[b] - even[b]
            eng.tensor_add(Rv[:, u, 1, j, :, co],
                           hv[:, 0 + j, :, :], hv[:, 2 + j, :, :])
            eng.tensor_sub(Rv[:, u, 2, j, :, co],
                           hv[:, 2 + j, :, :], hv[:, 0 + j, :, :])
            # R3[b] = even[b] - even[b+1]  (b<=14);  R3[15] = even[15]
            eng.tensor_sub(Rv[:, u, 3, j, 0:15, co],
                           hv[:, 0 + j, 0:15, :], hv[:, 0 + j, 1:16, :])
            eng.tensor_copy(Rv[:, u, 3, j, 15, co], hv[:, 0 + j, 15, :])

        def emit_V(Rv, vv, u, pc, eng):
            """Column transform (patch cols 2tc-1..2tc+2) for one u, one pc.
            j0 slice [0:16]=even[tc], [1:17]=even[tc+1];
            j1 slice [0:16]=odd[tc-1], [1:17]=odd[tc]."""
            for pr in range(4):
                p = 4 * pr + pc
                vdst = vv[:, p, u, :, :]
                def rsl(j, hi):
                    return Rv[:, u, pr, j, :, hi:hi + 16]
                if pc == 0:
                    eng.tensor_sub(vdst, rsl(1, 0), rsl(1, 1))
                elif pc == 1:
                    eng.tensor_add(vdst, rsl(0, 0), rsl(1, 1))
                elif pc == 2:
                    eng.tensor_sub(vdst, rsl(1, 1), rsl(0, 0))
                else:
                    eng.tensor_sub(vdst, rsl(0, 0), rsl(0, 1))

        hl_cur = None
        bo_prev = None
        R_cur = None
        vh = None

        for t in range(T):
            xv = xp.tile([128, 8192], F16, tag="xv")
            if t == 0:
                for pc in range(4):
                    for pr in range(4):
                        p = 4 * pr + pc
                        nc.gpsimd.dma_start(xv[:, p * 512:(p + 1) * 512],
                                            x_d[0][:, p * 512:(p + 1) * 512])
            else:
                nc.sync.dma_start(xv[:, 0:4096], x_d[t][:, 0:4096])
                nc.sync.dma_start(xv[:, 4096:8192], x_d[t][:, 4096:8192])

            if t == 1:
                # deferred h-conv weight stream (first-use order, both HWDGE
                # queues) — starts after the t=0 AllReduce is in flight
                for pc in range(4):
                    for pr in range(4):
                        for u in range(2):
                            j = 32 + (4 * pr + pc) * 2 + u
                            q = nc.sync if u == 0 else nc.scalar
                            q.dma_start(ws[j][:], w_d[j])

            if t > 0:
                # remote half: AR result in; h_rem = (own+pair) - own.
                # Critical prefix ordered j1-first then V pc0 so the first
                # remote h-matmuls unblock as early as possible.
                R, Rv, vv = R_cur
                hsum = sp.tile([128, 1024], F16, tag="hsum")
                nc.gpsimd.dma_start(hsum[:], bo_prev[:])
                hrem = sp.tile([128, 1024], F16, tag="hrem")
                nc.vector.tensor_sub(hrem[:], hsum[:], hl_cur[:])
                emit_R(Rv, 1, 1, hrem, nc.vector)
                emit_V(Rv, vv, 1, 0, nc.vector)
                emit_R(Rv, 1, 0, hrem, nc.vector)
                emit_V(Rv, vv, 1, 1, nc.vector)
                emit_V(Rv, vv, 1, 2, nc.gpsimd)
                emit_V(Rv, vv, 1, 3, nc.gpsimd)

            octs = [GI, GG, GO] if t == 0 else [GF, GI, GG, GO]
            waves = [(g, pc) for g in octs for pc in range(4)]

            ps_of = {}
            yp_of = {}

            def emit_x(w):
                g, pc = w
                ps = [pp.tile([128, 256], F32, tag=f"ps{r}", name=f"ps{r}")
                      for r in range(4)]
                ps_of[w] = (None, ps)
                for pr in range(4):
                    p = 4 * pr + pc
                    for u in range(2):
                        nc.tensor.matmul(
                            ps[pr][:], ws[p * 2 + u][:, g * 128:(g + 1) * 128],
                            xv[:, p * 512 + u * 256: p * 512 + u * 256 + 256],
                            start=(u == 0), stop=(t == 0 and u == 1),
                            skip_group_check=True)

            def emit_h(w):
                if t == 0:
                    return
                g, pc = w
                pst, ps = ps_of[w]
                # u=0 (local half, available early) before u=1 (post-AR)
                for u in range(2):
                    for pr in range(4):
                        p = 4 * pr + pc
                        nc.tensor.matmul(
                            ps[pr][:],
                            ws[32 + p * 2 + u][:, g * 128:(g + 1) * 128],
                            vh[:, p * 512 + u * 256: p * 512 + u * 256 + 256],
                            start=False, stop=(u == 1), skip_group_check=True)

            def emit_evac(w):
                g, pc = w
                pst, ps = ps_of[w]
                if g not in yp_of:
                    yp_of[g] = yp.tile([128, 2048], F16, tag="yp", name="ypg")
                ypg = yp_of[g]
                s = ep.tile([128, 512], F16, tag="s")
                nc.scalar.activation(s[:, 0:256], ps[1][:], AF.Copy)
                nc.scalar.activation(s[:, 256:512], ps[2][:], AF.Copy)
                y0 = ypg[:, pc * 256:(pc + 1) * 256]
                y1 = ypg[:, 1024 + pc * 256: 1024 + (pc + 1) * 256]
                # y'0 = M0+M1+M2 ; y'1 = M1-M2-M3
                nc.vector.tensor_add(y0, ps[0][:], s[:, 0:256])
                nc.vector.tensor_add(y0, y0, s[:, 256:512])
                nc.gpsimd.tensor_sub(y1, s[:, 0:256], s[:, 256:512])
                nc.vector.tensor_sub(y1, y1, ps[3][:])

            gt = gp.tile([128, 4096], F16, tag="gt")

            def emit_colact(g):
                ypg = yp_of[g]
                ypv = ypg[:].rearrange("p (i q c) -> p i q c", i=2, q=4, c=256)
                yt = sp.tile([128, 1024], F16, tag="yt")
                ytv = yt[:].rearrange("p (i j c) -> p i j c", i=2, j=2, c=256)
                # Y[:,0] = y'0+y'1+y'2 ; Y[:,1] = y'1-y'2-y'3  (over pc)
                nc.vector.tensor_add(ytv[:, :, 0, :], ypv[:, :, 0, :],
                                     ypv[:, :, 1, :])
                nc.vector.tensor_add(ytv[:, :, 0, :], ytv[:, :, 0, :],
                                     ypv[:, :, 2, :])
                nc.vector.tensor_sub(ytv[:, :, 1, :], ypv[:, :, 1, :],
                                     ypv[:, :, 2, :])
                nc.vector.tensor_sub(ytv[:, :, 1, :], ytv[:, :, 1, :],
                                     ypv[:, :, 3, :])
                gsl = gt[:, g * 1024:(g + 1) * 1024]
                if g == GG:
                    nc.vector.tensor_scalar(gsl, yt[:], bt[:, g:g + 1], 0.0,
                                            ALU.add, ALU.max)
                else:
                    nc.scalar.activation(gsl, yt[:], AF.Sigmoid,
                                         bias=bt[:, g:g + 1])

            prev = None
            for w in waves:
                emit_x(w)
                if prev is not None:
                    emit_h(prev)
                    emit_evac(prev)
                    if prev[1] == 3:
                        emit_colact(prev[0])
                prev = w
            emit_h(prev)
            emit_evac(prev)
            emit_colact(prev[0])

            # state update (quad layout [q*256 + band*16 + tc])
            g_i = gt[:, GI * 1024:(GI + 1) * 1024]
            g_f = gt[:, GF * 1024:(GF + 1) * 1024]
            g_o = gt[:, GO * 1024:(GO + 1) * 1024]
            g_g = gt[:, GG * 1024:(GG + 1) * 1024]
            cr = sp.tile([128, 1024], F16, tag="cr")
            if t == 0:
                nc.vector.tensor_mul(c_t[:], g_i, g_g)
            else:
                tig = sp.tile([128, 1024], F16, tag="tig")
                nc.vector.tensor_mul(tig[:], g_i, g_g)
                nc.vector.tensor_mul(c_t[:], g_f, c_t[:])
                nc.vector.tensor_add(c_t[:], c_t[:], tig[:])
            nc.vector.tensor_scalar_max(cr[:], c_t[:], 0.0)

            if t < T - 1:
                hl = sp.tile([128, 1024], F16, tag="hl")
                nc.vector.tensor_mul(hl[:], g_o, cr[:])
                bi = dp.tile([128, 1024], F16, tag="bi")
                bo = dp.tile([128, 1024], F16, tag="bo")
                nc.gpsimd.dma_start(bi[:], hl[:])
                nc.gpsimd.collective_compute(
                    "AllReduce", ALU.add, replica_groups=RG,
                    ins=[bi.opt()], outs=[bo.opt()])
                # local-half forward transform for step t+1 runs while the
                # AllReduce is in flight.
                Rn = rp.tile([128, 4352], F16, tag="R")
                Rvn = Rn[:].rearrange("p (u r j b c) -> p u r j b c",
                                      u=2, r=4, j=2, b=16, c=17)
                vh_n = vp.tile([128, 8192], F16, tag="vh")
                vvn = vh_n[:].rearrange("p (w u b c) -> p w u b c",
                                        w=16, u=2, b=16, c=16)
                emit_R(Rvn, 0, 1, hl, nc.vector)
                emit_R(Rvn, 0, 0, hl, nc.vector)
                for pc in range(4):
                    emit_V(Rvn, vvn, 0, pc, nc.gpsimd)
                R_cur = (Rn, Rvn, vvn)
                vh = vh_n
                hl_cur = hl
                bo_prev = bo
            else:
                ho = cp.tile([128, 1024], F32, tag="ho")
                nc.vector.tensor_mul(ho[:], g_o, cr[:])
                nc.sync.dma_start(o_d[:], ho[:])

    nc.compile()
    return nc


def _prep_U(wx, wh, s):
    """-> [64, 128, 512] f16; j = conv*32 + p*2 + u; cols = gate*128 + k.
    For the h-conv, ic-tile u=0 is the core's OWN hid slice (s), u=1 the
    pair's — matching the on-device local/remote V split."""
    out = np.empty((64, 128, 512), np.float16)
    for ci, w in enumerate((wx, wh)):
        U = np.einsum('pi,ocij,qj->ocpq', _G, np.asarray(w, np.float64), _G)
        U = U.reshape(4, 2, 128, 256, 4, 4)[:, s]       # [g, k, ic, pr, pc]
        U = U.transpose(3, 4, 2, 0, 1).reshape(16, 256, 512)
        U = U.reshape(16, 2, 128, 512)                  # [p, u, ic128, g*128+k]
        u0, u1 = (s, 1 - s) if ci == 1 else (0, 1)
        out[ci * 32 + 0: ci * 32 + 32: 2] = U[:, u0]
        out[ci * 32 + 1: ci * 32 + 32: 2] = U[:, u1]
    return out


def _prep_xv(xb):
    """xb [T,256,32,32] -> [T,128,8192] f16; free = p*512 + u*256 + tile."""
    xpad = np.zeros((T, 256, 34, 34), np.float32)
    xpad[:, :, 1:33, 1:33] = np.asarray(xb, np.float32)
    pt = np.lib.stride_tricks.sliding_window_view(
        xpad, (4, 4), axis=(2, 3))[:, :, ::2, ::2]       # [T,256,16,16,4,4]
    V = np.einsum('ij,tcrsjk,lk->tcrsil', _BT, pt.astype(np.float64), _BT)
    V = V.transpose(0, 1, 4, 5, 2, 3).reshape(T, 2, 128, 16, 256)
    V = V.transpose(0, 2, 3, 1, 4).reshape(T, 128, 8192)
    return np.ascontiguousarray(V).astype(np.float16)


def kernel(x, wx, wh, bh):
    x = np.asarray(x, dtype=np.float32)
    B = x.shape[0]
    bh = np.asarray(bh, np.float32).reshape(4, 2, 128)

    Us = [_prep_U(wx, wh, s) for s in range(2)]
    Xs = [_prep_xv(x[b]) for b in range(B)]

    in_maps = []
    for c in range(N_CORES):
        b, s = c // 2, c % 2
        in_maps.append({
            "w": Us[s],
            "xv": Xs[b],
            "bias": np.ascontiguousarray(bh[:, s].T),
        })

    if "nc" not in _cache:
        _cache["nc"] = _build_nc()
    nc = _cache["nc"]

    res = run_bass_kernel_spmd(nc, in_maps, core_ids=list(range(N_CORES)))
    _cache["last_results"] = res

    out = np.zeros((B, 256, 32, 32), dtype=np.float32)
    for c in range(N_CORES):
        b, s = c // 2, c % 2
        ho = res.results[c]["ho"].reshape(128, 2, 2, 16, 16)  # k, i, j, tr, tc
        out[b, s * 128:(s + 1) * 128] = (
            ho.transpose(0, 3, 1, 4, 2).reshape(128, 32, 32))
    return out
